# revision 18
# baseline (speedup 1.0000x reference)
"""Trainium2 Bass kernel for bidirectional cross-attention (nn_CrossAttention).

Reference computation (per batch b, N=1024 tokens, D=768 dims):
    sim1  = image1 @ image2^T            [N, N]
    out2  = l2norm(softmax(sim1) @ image2) + 2*image2
    sim2  = image2 @ image1^T
    out1  = l2norm(softmax(sim2) @ image1) + 2*image1

Key algebraic facts exploited:
  1. l2norm(softmax(S) @ V) == l2norm(exp(S - rowmax) @ V): the softmax
     denominator is a positive per-row scalar cancelled by the L2 norm.
  2. sim2 == sim1^T.  mm2 for direction 1 needs lhsT = P1^T[m,n]
     = exp(sim1^T[m,n] - M1[n]) -- i.e. exp applied to DIRECTION 2's mm1
     output with a bias along the FREE axis.  Symmetrically P2^T[n,m]
     = exp(sim1[n,m] - M2[m]).  So NO transposes of P are ever needed:
     each direction's exp reads the *other* direction's sim matrix.
     The free-axis bias is applied either by accumulating a broadcast
     (-M) into PSUM via 1-contraction-row "ones" matmuls (B side), or by
     a DVE tensor add against a broadcast tile (A side, from SBUF).
  3. All image tiles are pre-scaled by 2 (the residual is 2*img): the
     sims come out 4x too large, fixed by exp(scale=0.25); mm2's V is 2x
     too large, self-corrected because l2norm cancels the 2x and the
     epilogue needs (O*inv + 2*img) anyway.  This removes all residual
     re-reads from HBM (the old kernel re-read 25 MB/core).

Per-core structure (data parallel over batches, 2 per core):
  prep:  HBM f32 -> SBUF bf16 cast-DMA chunks -> DVE x2 -> resid2x;
         DMA xbar transpose (bf16) -> imgT16 -> cast-DMA -> imgT8 (fp8);
         cast-DMA resid2x -> V8 (fp8 natural).  Zero PE/ACT involvement.
  A:     per i: S_i = q1_i k2^T (fp8 DR matmuls) -> copy to Sb (fp16
         SBUF, ACT/DVE alternating) -> gpsimd rowmax -> -M1 column.
  B:     per j: ST_j = q2_j k1^T -> DVE rowmax (-M2 col) -> 8 ones-MM
         accumulate -M1 broadcast -> ACT exp(scale .25) -> P1T_j fp8.
  dir2:  X_i = Sb_i + (-M2 bcast) (DVE) -> ACT exp -> P2T_i fp8.
  mm2:   per g: O = P?T^T @ V8 (fp8 DR), sq+accum (ACT) -> ln -> exp
         -> inv; stt T3 = O*inv + resid2x (DVE); store (sync DMA).
All PSUM lives in one 4-slot x 2-bank pool; pipeline depth keeps every
engine busy without PSUM collisions.
"""

import os
import sys

import numpy as np

for _p in ("/opt/trn_rl_repo", "/root/.axon_site/_ro/trn_rl_repo"):
    if os.path.isdir(_p) and _p not in sys.path:
        sys.path.append(_p)

B, N, D = 16, 1024, 768
NCORES = 8
BPC = B // NCORES  # batches per core
P = 128
NT = N // P  # 8 token chunks
DT = D // P  # 6 feature chunks

_PROGRAM_CACHE = {}


def build_program():
    """Build the per-core Bass program (SPMD: identical on all cores)."""
    import concourse.mybir as mybir
    import concourse.tile as tile
    from concourse import bacc
    from concourse.masks import make_identity

    f32 = mybir.dt.float32
    bf16 = mybir.dt.bfloat16
    fp16 = mybir.dt.float16
    f8 = mybir.dt.float8e4
    AF = mybir.ActivationFunctionType
    ALU = mybir.AluOpType
    AX = mybir.AxisListType
    DR = mybir.MatmulPerfMode.DoubleRow

    nc = bacc.Bacc(None)

    # Keep Exp/Ln/Square/Copy/Identity resolvable from one ACT table set so
    # the table placement pass never bounces sets (each reload ~2.7us).
    from concourse.hw_specs import get_activation_tables

    _tabs = get_activation_tables(nc.m.arch)
    _keep = "natural_log_exp_and_others"
    if _keep in _tabs:
        _ours = {AF.Exp, AF.Ln, AF.Square, AF.Copy, AF.Identity}
        assert _ours <= _tabs[_keep]
        for _name, _s in _tabs.items():
            if _name != _keep:
                _s -= _ours

    img_dram = {
        1: nc.declare_dram_parameter("image1", [BPC, N, D], f32, isOutput=False),
        2: nc.declare_dram_parameter("image2", [BPC, N, D], f32, isOutput=False),
    }
    out_dram = {
        1: nc.declare_dram_parameter("out1", [BPC, N, D], f32, isOutput=True),
        2: nc.declare_dram_parameter("out2", [BPC, N, D], f32, isOutput=True),
    }

    with tile.TileContext(nc) as tc:
        with (
            tc.tile_pool(name="const", bufs=1) as const_pool,
            tc.tile_pool(name="imgs", bufs=2) as imgs,
            tc.tile_pool(name="ld", bufs=4) as ldp,
            tc.tile_pool(name="sb", bufs=1) as sbp,
            tc.tile_pool(name="xw", bufs=3) as xw,
            tc.tile_pool(name="outs", bufs=4) as outs,
            tc.tile_pool(name="junk", bufs=2) as junkp,
            tc.tile_pool(name="stats", bufs=8) as stats,
            tc.tile_pool(name="ps", bufs=4, space="PSUM") as ps,
        ):
            ident = const_pool.tile([P, P], bf16)
            make_identity(nc, ident[:])
            onescol = const_pool.tile([1, P], bf16)
            nc.vector.memset(onescol[:], 1.0)

            r2x = {}   # (b, im) -> [P, NT, D] bf16 2*img natural
            v8 = {}    # (b, im) -> [P, NT, D] f8   2*img natural
            t8 = {}    # (b, im) -> [P, DT, N] f8   (2*img)^T

            def prep_loads(b, im):
                """Load img f32 -> bf16 slabs, x2 on DVE, cast natural fp8.
                Returns closures that each PE-transpose one (dc) group and
                evacuate it to imgT8 (alternating ACT / DVE)."""
                r = imgs.tile([P, NT, D], bf16, tag=f"r2x{im}", name=f"r2x{im}")
                for h in range(2):  # two 4-chunk slabs per image
                    lc = ldp.tile([P, NT // 2, D], bf16, tag="ld", name="ldc")
                    src = img_dram[im][b, h * (N // 2) : (h + 1) * (N // 2), :]
                    nc.gpsimd.dma_start(lc[:], src.rearrange("(c p) d -> p c d", p=P))
                    nc.gpsimd.tensor_scalar_mul(
                        r[:, h * (NT // 2) : (h + 1) * (NT // 2), :], lc[:], 2.0
                    )
                v = imgs.tile([P, NT, D], f8, tag=f"v8{im}", name=f"v8{im}")
                nc.gpsimd.dma_start(v[:], r[:])
                t = imgs.tile([P, DT, N], f8, tag=f"t8{im}", name=f"t8{im}")
                r2x[(b, im)] = r
                v8[(b, im)] = v
                t8[(b, im)] = t

                def make(dc):
                    def g():
                        tp = ps.tile([P, NT, P], bf16, tag="ps", name="tp")
                        for c in range(NT):
                            nc.tensor.transpose(
                                tp[:, c, :], r[:, c, dc * P : (dc + 1) * P],
                                ident[:],
                            )
                        if dc % 2 == 0:
                            nc.scalar.activation(t[:, dc, :], tp[:], AF.Copy)
                        else:
                            nc.vector.tensor_copy(t[:, dc, :], tp[:])
                    return g

                return [make(dc) for dc in range(DT)]

            def prep_image(b, im):
                for g in prep_loads(b, im):
                    g()

            def mm1(S, qT, kT, i, stop=True):
                """S[:, :] (+)= q-chunk i of qT  @ kT   (fp8 DoubleRow)."""
                for dp in range(DT // 2):
                    lhsT = qT[:, 2 * dp : 2 * dp + 2, i * P : (i + 1) * P]
                    st = stop and dp == DT // 2 - 1
                    nc.tensor.matmul(
                        S[:, :512], lhsT, kT[:, 2 * dp : 2 * dp + 2, :512],
                        start=(dp == 0), stop=st, perf_mode=DR,
                    )
                    nc.tensor.matmul(
                        S[:, 512:], lhsT, kT[:, 2 * dp : 2 * dp + 2, 512:],
                        start=(dp == 0), stop=st, perf_mode=DR,
                    )

            def colvec_to_row(negcol):
                """[P, NT] per-partition column stats -> [1, N] bf16 SBUF row
                at partition 0 (for use as 1-row matmul rhs): one PE transpose
                per column lands all N values on partition 0; no DMA."""
                tpr = ps.tile([1, N], bf16, tag="ps", name="tpstat")
                for c in range(NT):
                    nc.tensor.transpose(
                        tpr[:, c * P : (c + 1) * P], negcol[:, c : c + 1], ident[:]
                    )
                mrow = stats.tile([1, N], bf16, tag="mrow", name="mrow", bufs=2)
                nc.scalar.activation(mrow[:], tpr[:], AF.Copy)
                return mrow

            def bias_rows(Spsum, mrow, start, stop):
                """Accumulate broadcast(-M) onto PSUM via 1-row ones-matmuls."""
                for h in range(2):
                    nc.tensor.matmul(
                        Spsum[:, h * 512 : (h + 1) * 512],
                        onescol[:], mrow[:, h * 512 : (h + 1) * 512],
                        start=start, stop=(stop and h == 1),
                        skip_group_check=True,
                    )

            def mm2_epilogue(b, O, im, g):
                sq = junkp.tile([P, D], bf16, tag="sq", name="sq")
                ss = stats.tile([P, 1], f32, tag="ss", name="ss")
                nc.scalar.activation(sq[:], O[:, :D], AF.Square, accum_out=ss[:])
                lss = stats.tile([P, 1], f32, tag="lss", name="lss")
                nc.scalar.activation(lss[:], ss[:], AF.Ln)
                inv = stats.tile([P, 1], f32, tag="inv", name="inv")
                nc.scalar.activation(inv[:], lss[:], AF.Exp, scale=-0.5)
                T3 = outs.tile([P, D], f32, tag="T3", name="T3")
                nc.vector.scalar_tensor_tensor(
                    out=T3[:], in0=O[:, :D], scalar=inv[:],
                    in1=r2x[(b, im)][:, g, :], op0=ALU.mult, op1=ALU.add,
                )
                nc.sync.dma_start(out_dram[im][b, g * P : (g + 1) * P, :], T3[:])

            def mm2_chunk(b, PT, im, g):
                """out[im] chunk g: O = PT^T @ V8 + l2norm epilogue + store."""
                V = v8[(b, im)]
                O = ps.tile([P, N], f32, tag="ps", name="O")
                for kp in range(NT // 2):
                    lhsT = PT[:, 2 * kp : 2 * kp + 2, g * P : (g + 1) * P]
                    nc.tensor.matmul(
                        O[:, :512], lhsT, V[:, 2 * kp : 2 * kp + 2, :512],
                        start=(kp == 0), stop=(kp == NT // 2 - 1), perf_mode=DR,
                    )
                    nc.tensor.matmul(
                        O[:, 512:D], lhsT, V[:, 2 * kp : 2 * kp + 2, 512:],
                        start=(kp == 0), stop=(kp == NT // 2 - 1), perf_mode=DR,
                    )
                mm2_epilogue(b, O, im, g)

            prep_image(0, 1)
            prep_image(0, 2)

            for b in range(BPC):
                q1T, q2T = t8[(b, 1)], t8[(b, 2)]

                # ---- A phase: S_i = q1_i @ k2^T; buffer fp16; gpsimd rowmax
                Sb = sbp.tile([P, NT, N], fp16, tag="Sb", name="Sb")
                negM1 = stats.tile([P, NT], bf16, tag="negM1", name="negM1", bufs=2)
                for i in range(NT):
                    S = ps.tile([P, N], f32, tag="ps", name="S")
                    mm1(S, q1T, q2T, i)
                    if i % 2 == 0:
                        nc.scalar.activation(Sb[:, i, :], S[:], AF.Copy)
                    else:
                        nc.vector.tensor_copy(Sb[:, i, :], S[:])
                    nc.vector.tensor_reduce(
                        negM1[:, i : i + 1], Sb[:, i, :], axis=AX.X, op=ALU.max,
                        negate=True,
                    )
                m1row = colvec_to_row(negM1)

                # ---- B phase: ST_j = q2_j @ k1^T; rowmax; -M1 bias; exp
                P1T = sbp.tile([P, NT, N], f8, tag="P1T", name="P1T")
                negM2 = stats.tile([P, NT], bf16, tag="negM2", name="negM2", bufs=2)
                pending = []
                for j in range(NT):
                    ST = ps.tile([P, N], f32, tag="ps", name="ST")
                    mm1(ST, q2T, q1T, j)
                    nc.vector.tensor_reduce(
                        negM2[:, j : j + 1], ST[:], axis=AX.X, op=ALU.max,
                        negate=True,
                    )
                    bias_rows(ST, m1row, start=False, stop=True)
                    nc.scalar.activation(P1T[:, j, :], ST[:], AF.Exp, scale=0.25)
                    # prefetch next batch's images early in the B phase,
                    # spreading the PE transpose groups across iterations
                    if j == 0 and b + 1 < BPC:
                        pending = prep_loads(b + 1, 1) + prep_loads(b + 1, 2)
                    if pending and j >= 1:
                        for g in pending[:2]:
                            g()
                        pending = pending[2:]
                m2row = colvec_to_row(negM2)
                for g in pending:
                    g()
                pending = []

                # -M2 broadcast tile for the A-side (dir2) exps
                mb = ps.tile([P, N], f32, tag="ps", name="mb")
                bias_rows(mb, m2row, start=True, stop=True)
                m2bc = xw.tile([P, N], fp16, tag="m2bc", name="m2bc", bufs=2)
                nc.scalar.activation(m2bc[:], mb[:], AF.Copy)

                # ---- dir2 P2T chain + mm2 dir1 interleaved
                P2T = sbp.tile([P, NT, N], f8, tag="P2T", name="P2T")

                def p2t_chain(i):
                    X = xw.tile([P, N], fp16, tag="X", name="X")
                    if i % 2 == 0:
                        nc.gpsimd.tensor_tensor(X[:], Sb[:, i, :], m2bc[:], op=ALU.add)
                    else:
                        nc.vector.tensor_tensor(X[:], Sb[:, i, :], m2bc[:], op=ALU.add)
                    nc.scalar.activation(P2T[:, i, :], X[:], AF.Exp, scale=0.25)

                p2t_chain(0)
                p2t_chain(1)
                # chunks 0,1 of mm2-dir1 accumulate j-outer: their first MMs
                # depend only on the early P1T tiles, so the PE rolls straight
                # from the B phase into mm2 without waiting for the last exp
                V2 = v8[(b, 2)]
                Os01 = [
                    ps.tile([P, N], f32, tag="ps", name="O"),
                    ps.tile([P, N], f32, tag="ps", name="O"),
                ]
                for kp in range(NT // 2):
                    for k in range(2):
                        O = Os01[k]
                        lhsT = P1T[:, 2 * kp : 2 * kp + 2, k * P : (k + 1) * P]
                        nc.tensor.matmul(
                            O[:, :512], lhsT, V2[:, 2 * kp : 2 * kp + 2, :512],
                            start=(kp == 0), stop=(kp == NT // 2 - 1), perf_mode=DR,
                        )
                        nc.tensor.matmul(
                            O[:, 512:D], lhsT, V2[:, 2 * kp : 2 * kp + 2, 512:],
                            start=(kp == 0), stop=(kp == NT // 2 - 1), perf_mode=DR,
                        )
                for k in range(2):
                    mm2_epilogue(b, Os01[k], 2, k)
                p2t_chain(2)
                p2t_chain(3)
                for g in range(2, NT):
                    mm2_chunk(b, P1T, 2, g)
                    if g + 2 < NT:
                        p2t_chain(g + 2)

                # ---- mm2 dir2
                for g in range(NT):
                    mm2_chunk(b, P2T, 1, g)

    return nc


def _get_program():
    if "nc" not in _PROGRAM_CACHE:
        nc = build_program()
        if not nc.is_finalized():
            nc.finalize()
        _PROGRAM_CACHE["nc"] = nc
    return _PROGRAM_CACHE["nc"]


def kernel(image1: np.ndarray, image2: np.ndarray):
    from concourse.bass_utils import run_bass_kernel_spmd

    image1 = np.ascontiguousarray(image1, dtype=np.float32)
    image2 = np.ascontiguousarray(image2, dtype=np.float32)
    assert image1.shape == (B, N, D) and image2.shape == (B, N, D)

    nc = _get_program()
    core_ids = list(range(NCORES))
    in_maps = [
        {
            "image1": image1[c * BPC : (c + 1) * BPC],
            "image2": image2[c * BPC : (c + 1) * BPC],
        }
        for c in core_ids
    ]
    res = run_bass_kernel_spmd(nc, in_maps, core_ids)
    out1 = np.concatenate([res.results[c]["out1"] for c in core_ids], axis=0)
    out2 = np.concatenate([res.results[c]["out2"] for c in core_ids], axis=0)
    return out1, out2


# revision 19
# speedup vs baseline: 2.6992x; 2.6992x over previous
"""Trainium2 Bass kernel for bidirectional cross-attention (nn_CrossAttention).

Reference computation (per batch b, N=1024 tokens, D=768 dims):
    sim1  = image1 @ image2^T            [N, N]
    out2  = l2norm(softmax(sim1) @ image2) + 2*image2
    sim2  = image2 @ image1^T
    out1  = l2norm(softmax(sim2) @ image1) + 2*image1

Key algebraic facts exploited:
  1. l2norm(softmax(S) @ V) == l2norm(exp(S - rowmax) @ V): the softmax
     denominator is a positive per-row scalar cancelled by the L2 norm.
  2. sim2 == sim1^T.  mm2 for direction 1 needs lhsT = P1^T[m,n]
     = exp(sim1^T[m,n] - M1[n]) -- i.e. exp applied to DIRECTION 2's mm1
     output with a bias along the FREE axis.  Symmetrically P2^T[n,m]
     = exp(sim1[n,m] - M2[m]).  So NO transposes of P are ever needed:
     each direction's exp reads the *other* direction's sim matrix.
     The free-axis bias is applied either by accumulating a broadcast
     (-M) into PSUM via 1-contraction-row "ones" matmuls (B side), or by
     a DVE tensor add against a broadcast tile (A side, from SBUF).
  3. All image tiles are pre-scaled by 2 (the residual is 2*img): the
     sims come out 4x too large, fixed by exp(scale=0.25); mm2's V is 2x
     too large, self-corrected because l2norm cancels the 2x and the
     epilogue needs (O*inv + 2*img) anyway.  This removes all residual
     re-reads from HBM (the old kernel re-read 25 MB/core).

Per-core structure (data parallel over batches, 2 per core):
  prep:  HBM f32 -> SBUF bf16 cast-DMA chunks -> DVE x2 -> resid2x;
         DMA xbar transpose (bf16) -> imgT16 -> cast-DMA -> imgT8 (fp8);
         cast-DMA resid2x -> V8 (fp8 natural).  Zero PE/ACT involvement.
  A:     per i: S_i = q1_i k2^T (fp8 DR matmuls) -> copy to Sb (fp16
         SBUF, ACT/DVE alternating) -> gpsimd rowmax -> -M1 column.
  B:     per j: ST_j = q2_j k1^T -> DVE rowmax (-M2 col) -> 8 ones-MM
         accumulate -M1 broadcast -> ACT exp(scale .25) -> P1T_j fp8.
  dir2:  X_i = Sb_i + (-M2 bcast) (DVE) -> ACT exp -> P2T_i fp8.
  mm2:   per g: O = P?T^T @ V8 (fp8 DR), sq+accum (ACT) -> ln -> exp
         -> inv; stt T3 = O*inv + resid2x (DVE); store (sync DMA).
All PSUM lives in one 4-slot x 2-bank pool; pipeline depth keeps every
engine busy without PSUM collisions.
"""

import os
import sys

import numpy as np

for _p in ("/opt/trn_rl_repo", "/root/.axon_site/_ro/trn_rl_repo"):
    if os.path.isdir(_p) and _p not in sys.path:
        sys.path.append(_p)

B, N, D = 16, 1024, 768
NCORES = 8
BPC = B // NCORES  # batches per core
P = 128
NT = N // P  # 8 token chunks
DT = D // P  # 6 feature chunks

_PROGRAM_CACHE = {}


def build_program():
    """Build the per-core Bass program (SPMD: identical on all cores)."""
    import concourse.mybir as mybir
    import concourse.tile as tile
    from concourse import bacc
    from concourse.masks import make_identity

    f32 = mybir.dt.float32
    bf16 = mybir.dt.bfloat16
    fp16 = mybir.dt.float16
    f8 = mybir.dt.float8e4
    AF = mybir.ActivationFunctionType
    ALU = mybir.AluOpType
    AX = mybir.AxisListType
    DR = mybir.MatmulPerfMode.DoubleRow

    nc = bacc.Bacc(None)

    # Keep Exp/Ln/Square/Copy/Identity resolvable from one ACT table set so
    # the table placement pass never bounces sets (each reload ~2.7us).
    from concourse.hw_specs import get_activation_tables

    _tabs = get_activation_tables(nc.m.arch)
    _keep = "natural_log_exp_and_others"
    if _keep in _tabs:
        _ours = {AF.Exp, AF.Ln, AF.Square, AF.Copy, AF.Identity}
        assert _ours <= _tabs[_keep]
        for _name, _s in _tabs.items():
            if _name != _keep:
                _s -= _ours

    img_dram = {
        1: nc.declare_dram_parameter("image1", [BPC, N, D], f32, isOutput=False),
        2: nc.declare_dram_parameter("image2", [BPC, N, D], f32, isOutput=False),
    }
    out_dram = {
        1: nc.declare_dram_parameter("out1", [BPC, N, D], f32, isOutput=True),
        2: nc.declare_dram_parameter("out2", [BPC, N, D], f32, isOutput=True),
    }

    with tile.TileContext(nc) as tc:
        with (
            tc.tile_pool(name="const", bufs=1) as const_pool,
            tc.tile_pool(name="imgs", bufs=2) as imgs,
            tc.tile_pool(name="ld", bufs=4) as ldp,
            tc.tile_pool(name="sb", bufs=1) as sbp,
            tc.tile_pool(name="xw", bufs=3) as xw,
            tc.tile_pool(name="outs", bufs=4) as outs,
            tc.tile_pool(name="junk", bufs=2) as junkp,
            tc.tile_pool(name="stats", bufs=8) as stats,
            tc.tile_pool(name="ps", bufs=4, space="PSUM") as ps,
        ):
            ident = const_pool.tile([P, P], bf16)
            make_identity(nc, ident[:])
            onescol = const_pool.tile([1, P], bf16)
            nc.vector.memset(onescol[:], 1.0)

            r2x = {}   # (b, im) -> [P, NT, D] bf16 2*img natural
            v8 = {}    # (b, im) -> [P, NT, D] f8   2*img natural
            t8 = {}    # (b, im) -> [P, DT, N] f8   (2*img)^T

            def prep_loads(b, im):
                """Load img f32 -> bf16 slabs, x2 on DVE, cast natural fp8.
                Returns closures that each PE-transpose one (dc) group and
                evacuate it to imgT8 (alternating ACT / DVE)."""
                r = imgs.tile([P, NT, D], bf16, tag=f"r2x{im}", name=f"r2x{im}")
                for h in range(2):  # two 4-chunk slabs per image
                    lc = ldp.tile([P, NT // 2, D], bf16, tag="ld", name="ldc")
                    src = img_dram[im][b, h * (N // 2) : (h + 1) * (N // 2), :]
                    nc.gpsimd.dma_start(lc[:], src.rearrange("(c p) d -> p c d", p=P))
                    nc.vector.tensor_scalar_mul(
                        r[:, h * (NT // 2) : (h + 1) * (NT // 2), :], lc[:], 2.0
                    )
                v = imgs.tile([P, NT, D], f8, tag=f"v8{im}", name=f"v8{im}")
                nc.gpsimd.dma_start(v[:], r[:])
                t = imgs.tile([P, DT, N], f8, tag=f"t8{im}", name=f"t8{im}")
                r2x[(b, im)] = r
                v8[(b, im)] = v
                t8[(b, im)] = t

                def make(dc):
                    def g():
                        tp = ps.tile([P, NT, P], bf16, tag="ps", name="tp")
                        for c in range(NT):
                            nc.tensor.transpose(
                                tp[:, c, :], r[:, c, dc * P : (dc + 1) * P],
                                ident[:],
                            )
                        if dc % 2 == 0:
                            nc.scalar.activation(t[:, dc, :], tp[:], AF.Copy)
                        else:
                            nc.vector.tensor_copy(t[:, dc, :], tp[:])
                    return g

                return [make(dc) for dc in range(DT)]

            def prep_image(b, im):
                for g in prep_loads(b, im):
                    g()

            def mm1(S, qT, kT, i, stop=True):
                """S[:, :] (+)= q-chunk i of qT  @ kT   (fp8 DoubleRow)."""
                for dp in range(DT // 2):
                    lhsT = qT[:, 2 * dp : 2 * dp + 2, i * P : (i + 1) * P]
                    st = stop and dp == DT // 2 - 1
                    nc.tensor.matmul(
                        S[:, :512], lhsT, kT[:, 2 * dp : 2 * dp + 2, :512],
                        start=(dp == 0), stop=st, perf_mode=DR,
                    )
                    nc.tensor.matmul(
                        S[:, 512:], lhsT, kT[:, 2 * dp : 2 * dp + 2, 512:],
                        start=(dp == 0), stop=st, perf_mode=DR,
                    )

            def colvec_to_row(negcol):
                """[P, NT] per-partition column stats -> [1, N] bf16 SBUF row
                at partition 0 (for use as 1-row matmul rhs): one PE transpose
                per column lands all N values on partition 0; no DMA."""
                tpr = ps.tile([1, N], bf16, tag="ps", name="tpstat")
                for c in range(NT):
                    nc.tensor.transpose(
                        tpr[:, c * P : (c + 1) * P], negcol[:, c : c + 1], ident[:]
                    )
                mrow = stats.tile([1, N], bf16, tag="mrow", name="mrow", bufs=2)
                nc.scalar.activation(mrow[:], tpr[:], AF.Copy)
                return mrow

            def bias_rows(Spsum, mrow, start, stop):
                """Accumulate broadcast(-M) onto PSUM via 1-row ones-matmuls."""
                for h in range(2):
                    nc.tensor.matmul(
                        Spsum[:, h * 512 : (h + 1) * 512],
                        onescol[:], mrow[:, h * 512 : (h + 1) * 512],
                        start=start, stop=(stop and h == 1),
                        skip_group_check=True,
                    )

            def mm2_epilogue(b, O, im, g):
                sq = junkp.tile([P, D], bf16, tag="sq", name="sq")
                ss = stats.tile([P, 1], f32, tag="ss", name="ss")
                nc.scalar.activation(sq[:], O[:, :D], AF.Square, accum_out=ss[:])
                lss = stats.tile([P, 1], f32, tag="lss", name="lss")
                nc.scalar.activation(lss[:], ss[:], AF.Ln)
                inv = stats.tile([P, 1], f32, tag="inv", name="inv")
                nc.scalar.activation(inv[:], lss[:], AF.Exp, scale=-0.5)
                T3 = outs.tile([P, D], f32, tag="T3", name="T3")
                nc.vector.scalar_tensor_tensor(
                    out=T3[:], in0=O[:, :D], scalar=inv[:],
                    in1=r2x[(b, im)][:, g, :], op0=ALU.mult, op1=ALU.add,
                )
                nc.sync.dma_start(out_dram[im][b, g * P : (g + 1) * P, :], T3[:])

            def mm2_chunk(b, PT, im, g):
                """out[im] chunk g: O = PT^T @ V8 + l2norm epilogue + store."""
                V = v8[(b, im)]
                O = ps.tile([P, N], f32, tag="ps", name="O")
                for kp in range(NT // 2):
                    lhsT = PT[:, 2 * kp : 2 * kp + 2, g * P : (g + 1) * P]
                    nc.tensor.matmul(
                        O[:, :512], lhsT, V[:, 2 * kp : 2 * kp + 2, :512],
                        start=(kp == 0), stop=(kp == NT // 2 - 1), perf_mode=DR,
                    )
                    nc.tensor.matmul(
                        O[:, 512:D], lhsT, V[:, 2 * kp : 2 * kp + 2, 512:],
                        start=(kp == 0), stop=(kp == NT // 2 - 1), perf_mode=DR,
                    )
                mm2_epilogue(b, O, im, g)

            prep_image(0, 1)
            prep_image(0, 2)

            for b in range(BPC):
                q1T, q2T = t8[(b, 1)], t8[(b, 2)]

                # ---- A phase: S_i = q1_i @ k2^T; buffer fp16; gpsimd rowmax
                Sb = sbp.tile([P, NT, N], fp16, tag="Sb", name="Sb")
                negM1 = stats.tile([P, NT], bf16, tag="negM1", name="negM1", bufs=2)
                for i in range(NT):
                    S = ps.tile([P, N], f32, tag="ps", name="S")
                    mm1(S, q1T, q2T, i)
                    if i % 2 == 0:
                        nc.scalar.activation(Sb[:, i, :], S[:], AF.Copy)
                    else:
                        nc.vector.tensor_copy(Sb[:, i, :], S[:])
                    nc.vector.tensor_reduce(
                        negM1[:, i : i + 1], Sb[:, i, :], axis=AX.X, op=ALU.max,
                        negate=True,
                    )
                m1row = colvec_to_row(negM1)

                # ---- B phase: ST_j = q2_j @ k1^T; rowmax; -M1 bias; exp
                P1T = sbp.tile([P, NT, N], f8, tag="P1T", name="P1T")
                negM2 = stats.tile([P, NT], bf16, tag="negM2", name="negM2", bufs=2)
                pending = []
                for j in range(NT):
                    ST = ps.tile([P, N], f32, tag="ps", name="ST")
                    mm1(ST, q2T, q1T, j)
                    nc.vector.tensor_reduce(
                        negM2[:, j : j + 1], ST[:], axis=AX.X, op=ALU.max,
                        negate=True,
                    )
                    bias_rows(ST, m1row, start=False, stop=True)
                    nc.scalar.activation(P1T[:, j, :], ST[:], AF.Exp, scale=0.25)
                    # prefetch next batch's images early in the B phase,
                    # spreading the PE transpose groups across iterations
                    if j == 0 and b + 1 < BPC:
                        pending = prep_loads(b + 1, 1) + prep_loads(b + 1, 2)
                    if pending and j >= 1:
                        for g in pending[:2]:
                            g()
                        pending = pending[2:]
                m2row = colvec_to_row(negM2)
                for g in pending:
                    g()
                pending = []

                # -M2 broadcast tile for the A-side (dir2) exps
                mb = ps.tile([P, N], f32, tag="ps", name="mb")
                bias_rows(mb, m2row, start=True, stop=True)
                m2bc = xw.tile([P, N], fp16, tag="m2bc", name="m2bc", bufs=2)
                nc.scalar.activation(m2bc[:], mb[:], AF.Copy)

                # ---- dir2 P2T chain + mm2 dir1 interleaved
                P2T = sbp.tile([P, NT, N], f8, tag="P2T", name="P2T")

                def p2t_chain(i):
                    X = xw.tile([P, N], fp16, tag="X", name="X")
                    if i % 2 == 0:
                        nc.gpsimd.tensor_tensor(X[:], Sb[:, i, :], m2bc[:], op=ALU.add)
                    else:
                        nc.vector.tensor_tensor(X[:], Sb[:, i, :], m2bc[:], op=ALU.add)
                    nc.scalar.activation(P2T[:, i, :], X[:], AF.Exp, scale=0.25)

                p2t_chain(0)
                p2t_chain(1)
                # chunks 0,1 of mm2-dir1 accumulate j-outer: their first MMs
                # depend only on the early P1T tiles, so the PE rolls straight
                # from the B phase into mm2 without waiting for the last exp
                V2 = v8[(b, 2)]
                Os01 = [
                    ps.tile([P, N], f32, tag="ps", name="O"),
                    ps.tile([P, N], f32, tag="ps", name="O"),
                ]
                for kp in range(NT // 2):
                    for k in range(2):
                        O = Os01[k]
                        lhsT = P1T[:, 2 * kp : 2 * kp + 2, k * P : (k + 1) * P]
                        nc.tensor.matmul(
                            O[:, :512], lhsT, V2[:, 2 * kp : 2 * kp + 2, :512],
                            start=(kp == 0), stop=(kp == NT // 2 - 1), perf_mode=DR,
                        )
                        nc.tensor.matmul(
                            O[:, 512:D], lhsT, V2[:, 2 * kp : 2 * kp + 2, 512:],
                            start=(kp == 0), stop=(kp == NT // 2 - 1), perf_mode=DR,
                        )
                for k in range(2):
                    mm2_epilogue(b, Os01[k], 2, k)
                p2t_chain(2)
                p2t_chain(3)
                for g in range(2, NT):
                    mm2_chunk(b, P1T, 2, g)
                    if g + 2 < NT:
                        p2t_chain(g + 2)

                # ---- mm2 dir2
                for g in range(NT):
                    mm2_chunk(b, P2T, 1, g)

    return nc


def _get_program():
    if "nc" not in _PROGRAM_CACHE:
        nc = build_program()
        if not nc.is_finalized():
            nc.finalize()
        _PROGRAM_CACHE["nc"] = nc
    return _PROGRAM_CACHE["nc"]


def kernel(image1: np.ndarray, image2: np.ndarray):
    from concourse.bass_utils import run_bass_kernel_spmd

    image1 = np.ascontiguousarray(image1, dtype=np.float32)
    image2 = np.ascontiguousarray(image2, dtype=np.float32)
    assert image1.shape == (B, N, D) and image2.shape == (B, N, D)

    nc = _get_program()
    core_ids = list(range(NCORES))
    in_maps = [
        {
            "image1": image1[c * BPC : (c + 1) * BPC],
            "image2": image2[c * BPC : (c + 1) * BPC],
        }
        for c in core_ids
    ]
    res = run_bass_kernel_spmd(nc, in_maps, core_ids)
    out1 = np.concatenate([res.results[c]["out1"] for c in core_ids], axis=0)
    out2 = np.concatenate([res.results[c]["out2"] for c in core_ids], axis=0)
    return out1, out2


# revision 20
# speedup vs baseline: 2.7721x; 1.0270x over previous
"""Trainium2 Bass kernel for bidirectional cross-attention (nn_CrossAttention).

Reference computation (per batch b, N=1024 tokens, D=768 dims):
    sim1  = image1 @ image2^T            [N, N]
    out2  = l2norm(softmax(sim1) @ image2) + 2*image2
    sim2  = image2 @ image1^T
    out1  = l2norm(softmax(sim2) @ image1) + 2*image1

Key algebraic facts exploited:
  1. l2norm(softmax(S) @ V) == l2norm(exp(S - rowmax) @ V): the softmax
     denominator is a positive per-row scalar cancelled by the L2 norm.
  2. sim2 == sim1^T.  mm2 for direction 1 needs lhsT = P1^T[m,n]
     = exp(sim1^T[m,n] - M1[n]) -- i.e. exp applied to DIRECTION 2's mm1
     output with a bias along the FREE axis.  Symmetrically P2^T[n,m]
     = exp(sim1[n,m] - M2[m]).  So NO transposes of P are ever needed:
     each direction's exp reads the *other* direction's sim matrix.
     The free-axis bias is applied either by accumulating a broadcast
     (-M) into PSUM via 1-contraction-row "ones" matmuls (B side), or by
     a DVE tensor add against a broadcast tile (A side, from SBUF).
  3. All image tiles are pre-scaled by 2 (the residual is 2*img): the
     sims come out 4x too large, fixed by exp(scale=0.25); mm2's V is 2x
     too large, self-corrected because l2norm cancels the 2x and the
     epilogue needs (O*inv + 2*img) anyway.  This removes all residual
     re-reads from HBM (the old kernel re-read 25 MB/core).

Per-core structure (data parallel over batches, 2 per core):
  prep:  HBM f32 -> SBUF bf16 cast-DMA chunks -> DVE x2 -> resid2x;
         DMA xbar transpose (bf16) -> imgT16 -> cast-DMA -> imgT8 (fp8);
         cast-DMA resid2x -> V8 (fp8 natural).  Zero PE/ACT involvement.
  A:     per i: S_i = q1_i k2^T (fp8 DR matmuls) -> copy to Sb (fp16
         SBUF, ACT/DVE alternating) -> gpsimd rowmax -> -M1 column.
  B:     per j: ST_j = q2_j k1^T -> DVE rowmax (-M2 col) -> 8 ones-MM
         accumulate -M1 broadcast -> ACT exp(scale .25) -> P1T_j fp8.
  dir2:  X_i = Sb_i + (-M2 bcast) (DVE) -> ACT exp -> P2T_i fp8.
  mm2:   per g: O = P?T^T @ V8 (fp8 DR), sq+accum (ACT) -> ln -> exp
         -> inv; stt T3 = O*inv + resid2x (DVE); store (sync DMA).
All PSUM lives in one 4-slot x 2-bank pool; pipeline depth keeps every
engine busy without PSUM collisions.
"""

import os
import sys

import numpy as np

for _p in ("/opt/trn_rl_repo", "/root/.axon_site/_ro/trn_rl_repo"):
    if os.path.isdir(_p) and _p not in sys.path:
        sys.path.append(_p)

B, N, D = 16, 1024, 768
NCORES = 8
BPC = B // NCORES  # batches per core
P = 128
NT = N // P  # 8 token chunks
DT = D // P  # 6 feature chunks

_PROGRAM_CACHE = {}


def build_program():
    """Build the per-core Bass program (SPMD: identical on all cores)."""
    import concourse.mybir as mybir
    import concourse.tile as tile
    from concourse import bacc
    from concourse.masks import make_identity

    f32 = mybir.dt.float32
    bf16 = mybir.dt.bfloat16
    fp16 = mybir.dt.float16
    f8 = mybir.dt.float8e4
    AF = mybir.ActivationFunctionType
    ALU = mybir.AluOpType
    AX = mybir.AxisListType
    DR = mybir.MatmulPerfMode.DoubleRow

    nc = bacc.Bacc(None)

    # Keep Exp/Ln/Square/Copy/Identity resolvable from one ACT table set so
    # the table placement pass never bounces sets (each reload ~2.7us).
    from concourse.hw_specs import get_activation_tables

    _tabs = get_activation_tables(nc.m.arch)
    _keep = "natural_log_exp_and_others"
    if _keep in _tabs:
        _ours = {AF.Exp, AF.Ln, AF.Square, AF.Copy, AF.Identity}
        assert _ours <= _tabs[_keep]
        for _name, _s in _tabs.items():
            if _name != _keep:
                _s -= _ours

    img_dram = {
        1: nc.declare_dram_parameter("image1", [BPC, N, D], f32, isOutput=False),
        2: nc.declare_dram_parameter("image2", [BPC, N, D], f32, isOutput=False),
    }
    out_dram = {
        1: nc.declare_dram_parameter("out1", [BPC, N, D], f32, isOutput=True),
        2: nc.declare_dram_parameter("out2", [BPC, N, D], f32, isOutput=True),
    }

    with tile.TileContext(nc) as tc:
        with (
            tc.tile_pool(name="const", bufs=1) as const_pool,
            tc.tile_pool(name="imgs", bufs=2) as imgs,
            tc.tile_pool(name="ld", bufs=4) as ldp,
            tc.tile_pool(name="sb", bufs=1) as sbp,
            tc.tile_pool(name="xw", bufs=3) as xw,
            tc.tile_pool(name="outs", bufs=4) as outs,
            tc.tile_pool(name="junk", bufs=2) as junkp,
            tc.tile_pool(name="stats", bufs=8) as stats,
            tc.tile_pool(name="ps", bufs=4, space="PSUM") as ps,
        ):
            ident = const_pool.tile([P, P], bf16)
            make_identity(nc, ident[:])
            onescol = const_pool.tile([1, P], bf16)
            nc.vector.memset(onescol[:], 1.0)

            r2x = {}   # (b, im) -> [P, NT, D] bf16 2*img natural
            v8 = {}    # (b, im) -> [P, NT, D] f8   2*img natural
            t8 = {}    # (b, im) -> [P, DT, N] f8   (2*img)^T

            def prep_loads(b, im):
                """Load img f32 -> bf16 slabs, x2 on DVE, cast natural fp8.
                Returns closures that each PE-transpose one (dc) group and
                evacuate it to imgT8 (alternating ACT / DVE)."""
                r = imgs.tile([P, NT, D], bf16, tag=f"r2x{im}", name=f"r2x{im}")
                for h in range(2):  # two 4-chunk slabs per image
                    lc = ldp.tile([P, NT // 2, D], bf16, tag="ld", name="ldc")
                    src = img_dram[im][b, h * (N // 2) : (h + 1) * (N // 2), :]
                    nc.gpsimd.dma_start(lc[:], src.rearrange("(c p) d -> p c d", p=P))
                    nc.vector.tensor_scalar_mul(
                        r[:, h * (NT // 2) : (h + 1) * (NT // 2), :], lc[:], 2.0
                    )
                v = imgs.tile([P, NT, D], f8, tag=f"v8{im}", name=f"v8{im}")
                nc.gpsimd.dma_start(v[:], r[:])
                t = imgs.tile([P, DT, N], f8, tag=f"t8{im}", name=f"t8{im}")
                r2x[(b, im)] = r
                v8[(b, im)] = v
                t8[(b, im)] = t

                def make(dc):
                    def g():
                        tp = ps.tile([P, NT, P], bf16, tag="ps", name="tp")
                        for c in range(NT):
                            nc.tensor.transpose(
                                tp[:, c, :], r[:, c, dc * P : (dc + 1) * P],
                                ident[:],
                            )
                        if dc % 2 == 0:
                            nc.scalar.activation(t[:, dc, :], tp[:], AF.Copy)
                        else:
                            nc.vector.tensor_copy(t[:, dc, :], tp[:])
                    return g

                return [make(dc) for dc in range(DT)]

            def prep_image(b, im):
                for g in prep_loads(b, im):
                    g()

            def mm1(S, qT, kT, i, stop=True):
                """S[:, :] (+)= q-chunk i of qT  @ kT   (fp8 DoubleRow)."""
                for dp in range(DT // 2):
                    lhsT = qT[:, 2 * dp : 2 * dp + 2, i * P : (i + 1) * P]
                    st = stop and dp == DT // 2 - 1
                    nc.tensor.matmul(
                        S[:, :512], lhsT, kT[:, 2 * dp : 2 * dp + 2, :512],
                        start=(dp == 0), stop=st, perf_mode=DR,
                    )
                    nc.tensor.matmul(
                        S[:, 512:], lhsT, kT[:, 2 * dp : 2 * dp + 2, 512:],
                        start=(dp == 0), stop=st, perf_mode=DR,
                    )

            def colvec_to_row(negcol):
                """[P, NT] per-partition column stats -> [1, N] bf16 SBUF row
                at partition 0 (for use as 1-row matmul rhs): one PE transpose
                per column lands all N values on partition 0; no DMA."""
                tpr = ps.tile([1, N], bf16, tag="ps", name="tpstat")
                for c in range(NT):
                    nc.tensor.transpose(
                        tpr[:, c * P : (c + 1) * P], negcol[:, c : c + 1], ident[:]
                    )
                mrow = stats.tile([1, N], bf16, tag="mrow", name="mrow", bufs=2)
                nc.scalar.activation(mrow[:], tpr[:], AF.Copy)
                return mrow

            def bias_rows(Spsum, mrow, start, stop):
                """Accumulate broadcast(-M) onto PSUM via 1-row ones-matmuls."""
                for h in range(2):
                    nc.tensor.matmul(
                        Spsum[:, h * 512 : (h + 1) * 512],
                        onescol[:], mrow[:, h * 512 : (h + 1) * 512],
                        start=start, stop=(stop and h == 1),
                        skip_group_check=True,
                    )

            def mm2_epilogue(b, O, im, g):
                sq = junkp.tile([P, D], bf16, tag="sq", name="sq")
                ss = stats.tile([P, 1], f32, tag="ss", name="ss")
                nc.scalar.activation(sq[:], O[:, :D], AF.Square, accum_out=ss[:])
                lss = stats.tile([P, 1], f32, tag="lss", name="lss")
                nc.scalar.activation(lss[:], ss[:], AF.Ln)
                inv = stats.tile([P, 1], f32, tag="inv", name="inv")
                nc.scalar.activation(inv[:], lss[:], AF.Exp, scale=-0.5)
                T3 = outs.tile([P, D], f32, tag="T3", name="T3")
                nc.vector.scalar_tensor_tensor(
                    out=T3[:], in0=O[:, :D], scalar=inv[:],
                    in1=r2x[(b, im)][:, g, :], op0=ALU.mult, op1=ALU.add,
                )
                nc.sync.dma_start(out_dram[im][b, g * P : (g + 1) * P, :], T3[:])

            def mm2_chunk(b, PT, im, g):
                """out[im] chunk g: O = PT^T @ V8 + l2norm epilogue + store."""
                V = v8[(b, im)]
                O = ps.tile([P, N], f32, tag="ps", name="O")
                for kp in range(NT // 2):
                    lhsT = PT[:, 2 * kp : 2 * kp + 2, g * P : (g + 1) * P]
                    nc.tensor.matmul(
                        O[:, :512], lhsT, V[:, 2 * kp : 2 * kp + 2, :512],
                        start=(kp == 0), stop=(kp == NT // 2 - 1), perf_mode=DR,
                    )
                    nc.tensor.matmul(
                        O[:, 512:D], lhsT, V[:, 2 * kp : 2 * kp + 2, 512:],
                        start=(kp == 0), stop=(kp == NT // 2 - 1), perf_mode=DR,
                    )
                mm2_epilogue(b, O, im, g)

            prep_image(0, 1)
            prep_image(0, 2)

            for b in range(BPC):
                q1T, q2T = t8[(b, 1)], t8[(b, 2)]

                # ---- A phase: S_i = q1_i @ k2^T; buffer fp16; gpsimd rowmax
                Sb = sbp.tile([P, NT, N], fp16, tag="Sb", name="Sb")
                negM1 = stats.tile([P, NT], bf16, tag="negM1", name="negM1", bufs=2)
                for i in range(NT):
                    S = ps.tile([P, N], f32, tag="ps", name="S")
                    mm1(S, q1T, q2T, i)
                    if i % 2 == 0:
                        nc.scalar.activation(Sb[:, i, :], S[:], AF.Copy)
                    else:
                        nc.vector.tensor_copy(Sb[:, i, :], S[:])
                    nc.vector.tensor_reduce(
                        negM1[:, i : i + 1], Sb[:, i, :], axis=AX.X, op=ALU.max,
                        negate=True,
                    )
                m1row = colvec_to_row(negM1)

                # ---- B phase: ST_j = q2_j @ k1^T; rowmax; -M1 bias; exp
                P1T = sbp.tile([P, NT, N], f8, tag="P1T", name="P1T")
                negM2 = stats.tile([P, NT], bf16, tag="negM2", name="negM2", bufs=2)
                pending = []
                for j in range(NT):
                    ST = ps.tile([P, N], f32, tag="ps", name="ST")
                    mm1(ST, q2T, q1T, j)
                    nc.vector.tensor_reduce(
                        negM2[:, j : j + 1], ST[:], axis=AX.X, op=ALU.max,
                        negate=True,
                    )
                    bias_rows(ST, m1row, start=False, stop=True)
                    nc.scalar.activation(P1T[:, j, :], ST[:], AF.Exp, scale=0.25)
                    # prefetch next batch's images early in the B phase,
                    # spreading the PE transpose groups across iterations
                    if j == 0 and b + 1 < BPC:
                        pending = prep_loads(b + 1, 1) + prep_loads(b + 1, 2)
                    if pending and j >= 1 and j <= 4:
                        for g in pending[:2]:
                            g()
                        pending = pending[2:]
                m2row = colvec_to_row(negM2)
                # fill the B->mm2 seam (mm2 waits on the last P1T exp) with
                # the remaining prefetch transpose groups
                for g in pending:
                    g()
                pending = []

                # -M2 broadcast tile for the A-side (dir2) exps
                mb = ps.tile([P, N], f32, tag="ps", name="mb")
                bias_rows(mb, m2row, start=True, stop=True)
                m2bc = xw.tile([P, N], fp16, tag="m2bc", name="m2bc", bufs=2)
                nc.scalar.activation(m2bc[:], mb[:], AF.Copy)

                # ---- dir2 P2T chain + mm2 dir1 interleaved
                P2T = sbp.tile([P, NT, N], f8, tag="P2T", name="P2T")

                def p2t_chain(i):
                    X = xw.tile([P, N], fp16, tag="X", name="X")
                    if i % 2 == 0:
                        nc.gpsimd.tensor_tensor(X[:], Sb[:, i, :], m2bc[:], op=ALU.add)
                    else:
                        nc.vector.tensor_tensor(X[:], Sb[:, i, :], m2bc[:], op=ALU.add)
                    nc.scalar.activation(P2T[:, i, :], X[:], AF.Exp, scale=0.25)

                p2t_chain(0)
                p2t_chain(1)
                for g0 in range(0, NT, 2):
                    mm2_chunk(b, P1T, 2, g0)
                    mm2_chunk(b, P1T, 2, g0 + 1)
                    for i in (g0 + 2, g0 + 3):
                        if i < NT:
                            p2t_chain(i)

                # ---- mm2 dir2
                for g in range(NT):
                    mm2_chunk(b, P2T, 1, g)

    return nc


def _get_program():
    if "nc" not in _PROGRAM_CACHE:
        nc = build_program()
        if not nc.is_finalized():
            nc.finalize()
        _PROGRAM_CACHE["nc"] = nc
    return _PROGRAM_CACHE["nc"]


def kernel(image1: np.ndarray, image2: np.ndarray):
    from concourse.bass_utils import run_bass_kernel_spmd

    image1 = np.ascontiguousarray(image1, dtype=np.float32)
    image2 = np.ascontiguousarray(image2, dtype=np.float32)
    assert image1.shape == (B, N, D) and image2.shape == (B, N, D)

    nc = _get_program()
    core_ids = list(range(NCORES))
    in_maps = [
        {
            "image1": image1[c * BPC : (c + 1) * BPC],
            "image2": image2[c * BPC : (c + 1) * BPC],
        }
        for c in core_ids
    ]
    res = run_bass_kernel_spmd(nc, in_maps, core_ids)
    out1 = np.concatenate([res.results[c]["out1"] for c in core_ids], axis=0)
    out2 = np.concatenate([res.results[c]["out2"] for c in core_ids], axis=0)
    return out1, out2
